# revision 13
# baseline (speedup 1.0000x reference)
"""Trainium2 Bass kernel for nn_Attention_52982716563627.

Module: qkv = x@W_atten + b_atten; per-head scores s = q k^T (no scaling);
mask applied as w*mask + (1-mask)*1e5; softmax over the HEAD axis (quirk!);
a = p @ v; out = a @ W_proj + b_proj.

Key identity: softmax is over heads at each (i,j). Masked entries (+1e5 for
all 16 heads) softmax to exactly 1/16. Scores are bounded (|s| < ~40), so
max-subtraction can be skipped: set masked scores to 0 -> exp=1 -> uniform
1/16 falls out of the normalization automatically:
    e = exp(s * causal_mask);  Z = sum_h e;  p = e / Z;  a = p @ v
which matches the reference exactly (softmax is shift-invariant).

Sharding: 8 cores = 2 batches x 4 query-blocks of 512 rows (SPMD, one
program; all per-core differences enter via input data: xTq slice + mask).
K/V are computed on every core of a batch group from the full x[b]; v is
bounced through DRAM to fit SBUF.

Layouts (per core):
  kT resident [128p=(h%2)*64+d, 8=h//2, 2048 keys]     (head-major cols)
  qT resident [128, 8, 512 own q rows]
  v in DRAM   [2048 keys, 1024 cols head-major]
  scores computed transposed: sT[key, q] so AV needs no transposes; K=64
  score matmuls on even/odd partition halves pack 2x via PE 64x128 tiling.
  e tiles [128 keys, 16 h, 256 q]; Z = chain-sum over h; p = e * (1/Z).
  AV accumulates over 16 key chunks into 4 PSUM banks (4 heads per bank:
  2 on partition halves x 2 on free halves).
"""

import numpy as np

import concourse.bacc as bacc
import concourse.mybir as mybir
import concourse.tile as tile
from concourse import bass_utils

N_CORES = 8
B, S, E = 2, 2048, 1024
H, HD = 16, 64
DQ = 512   # q rows per core
QH = 256   # q tile (half block)
KC = 128   # key chunk
NKC = S // KC  # 16
FP = mybir.dt.float32
AF = mybir.ActivationFunctionType


def build_program(reps: int = 1, debug_taps: bool = False):
    nc = bacc.Bacc("TRN2", target_bir_lowering=False, debug=False,
                   num_devices=N_CORES)
    if debug_taps:
        dbg_kt = nc.dram_tensor("dbg_kt", [128, 8, S], FP,
                                kind="ExternalOutput")
        dbg_qt = nc.dram_tensor("dbg_qt", [128, 8, DQ], FP,
                                kind="ExternalOutput")
        dbg_eb = nc.dram_tensor("dbg_eb", [128, H, QH], FP,
                                kind="ExternalOutput")
        dbg_at = nc.dram_tensor("dbg_at", [128, 8, QH], FP,
                                kind="ExternalOutput")
        dbg_p = nc.dram_tensor("dbg_p", [128, H, QH], FP,
                               kind="ExternalOutput")

    xT_d = nc.dram_tensor("xT", [E, S], FP, kind="ExternalInput")
    xTq_d = nc.dram_tensor("xTq", [E, DQ], FP, kind="ExternalInput")
    wq_d = nc.dram_tensor("wq", [E, E], FP, kind="ExternalInput")
    wk_d = nc.dram_tensor("wk", [E, E], FP, kind="ExternalInput")
    wv_d = nc.dram_tensor("wv", [E, E], FP, kind="ExternalInput")
    wo_d = nc.dram_tensor("wo", [E, E], FP, kind="ExternalInput")
    bq_d = nc.dram_tensor("bq", [128, 8], FP, kind="ExternalInput")
    bk_d = nc.dram_tensor("bk", [128, 8], FP, kind="ExternalInput")
    bv_d = nc.dram_tensor("bv", [1, E], FP, kind="ExternalInput")
    bo_d = nc.dram_tensor("bo", [1, E], FP, kind="ExternalInput")
    mask_d = nc.dram_tensor("maskT", [NKC, KC, DQ], FP, kind="ExternalInput")
    out_d = nc.dram_tensor("out", [DQ, E], FP, kind="ExternalOutput")

    with tile.TileContext(nc) as tc:
        with (
            tc.tile_pool(name="consts", bufs=1) as consts,
            tc.tile_pool(name="kt", bufs=1) as ktp,
            tc.tile_pool(name="qt", bufs=1) as qtp,
            tc.tile_pool(name="vdram", bufs=1, space="DRAM") as vdp,
        ):
            ones_sb = consts.tile([1, 128], FP)
            nc.vector.memset(ones_sb[:], 1.0)
            zeros_sb = consts.tile([1, 512], FP)
            nc.vector.memset(zeros_sb[:], 0.0)
            bq_sb = consts.tile([128, 8], FP)
            bk_sb = consts.tile([128, 8], FP)
            bv_sb = consts.tile([1, E], FP)
            bo_sb = consts.tile([1, E], FP)
            nc.sync.dma_start(bq_sb[:], bq_d[:])
            nc.sync.dma_start(bk_sb[:], bk_d[:])
            nc.sync.dma_start(bv_sb[:], bv_d[:])
            nc.sync.dma_start(bo_sb[:], bo_d[:])

            kt = ktp.tile([128, 8, S], FP)
            qt = qtp.tile([128, 8, DQ], FP)
            v_dram = vdp.tile([S, E], FP)

            loop = tc.For_i(0, reps, 1) if reps > 1 else None
            if loop is not None:
                loop.__enter__()

            # ---------------- Phase A1: qT and v ----------------
            with (
                tc.tile_pool(name="xt", bufs=1) as xtp,
                tc.tile_pool(name="xtq", bufs=1) as xtqp,
                tc.tile_pool(name="wstr", bufs=2) as wstr,
                tc.tile_pool(name="wvp", bufs=1) as wvp,
                tc.tile_pool(name="vout", bufs=2) as voutp,
                tc.tile_pool(name="pskq", bufs=2, space="PSUM") as pskq,
                tc.tile_pool(name="psv", bufs=2, space="PSUM") as psv,
            ):
                xt = xtp.tile([128, 8, S], FP)
                for e in range(8):
                    nc.sync.dma_start(
                        xt[:, e, :], xT_d[e * 128:(e + 1) * 128, :])
                xtq = xtqp.tile([128, 8, DQ], FP)
                for e in range(8):
                    nc.sync.dma_start(
                        xtq[:, e, :], xTq_d[e * 128:(e + 1) * 128, :])

                # qT: out.T orientation [cols, rows]; bias per-partition
                for ct in range(8):
                    wct = wstr.tile([128, 8, 128], FP, tag="wct")
                    nc.sync.dma_start(
                        wct[:],
                        wq_d[:, ct * 128:(ct + 1) * 128].rearrange(
                            "(e p) c -> p e c", p=128))
                    ps = pskq.tile([128, DQ], FP)
                    for e in range(8):
                        nc.tensor.matmul(ps[:], wct[:, e, :], xtq[:, e, :],
                                         start=(e == 0), stop=(e == 7))
                    nc.scalar.activation(qt[:, ct, :], ps[:], AF.Identity,
                                         bias=bq_sb[:, ct:ct + 1])

                # v natural [keys, cols]: bias via K=1 ones matmul
                for cc in range(2):
                    wvcc = wvp.tile([128, 8, 512], FP, tag="wvcc")
                    nc.sync.dma_start(
                        wvcc[:],
                        wv_d[:, cc * 512:(cc + 1) * 512].rearrange(
                            "(e p) c -> p e c", p=128))
                    for rt in range(16):
                        ps = psv.tile([128, 512], FP)
                        for e in range(8):
                            nc.tensor.matmul(
                                ps[:], xt[:, e, rt * 128:(rt + 1) * 128],
                                wvcc[:, e, :], start=(e == 0), stop=False)
                        nc.tensor.matmul(ps[:], ones_sb[:1, :],
                                         bv_sb[:1, cc * 512:(cc + 1) * 512],
                                         start=False, stop=True)
                        vo = voutp.tile([128, 512], FP, tag="vo")
                        nc.scalar.activation(vo[:], ps[:], AF.Copy)
                        nc.sync.dma_start(
                            v_dram[rt * 128:(rt + 1) * 128,
                                   cc * 512:(cc + 1) * 512], vo[:])

                # kT (xt still resident)
                for ct in range(8):
                    wct = wstr.tile([128, 8, 128], FP, tag="wct")
                    nc.sync.dma_start(
                        wct[:],
                        wk_d[:, ct * 128:(ct + 1) * 128].rearrange(
                            "(e p) c -> p e c", p=128))
                    for kc4 in range(4):
                        ps = pskq.tile([128, DQ], FP)
                        for e in range(8):
                            nc.tensor.matmul(
                                ps[:], wct[:, e, :],
                                xt[:, e, kc4 * 512:(kc4 + 1) * 512],
                                start=(e == 0), stop=(e == 7))
                        nc.scalar.activation(
                            kt[:, ct, kc4 * 512:(kc4 + 1) * 512], ps[:],
                            AF.Identity, bias=bk_sb[:, ct:ct + 1])

            if debug_taps:
                nc.sync.dma_start(dbg_kt[:], kt[:])
                nc.sync.dma_start(dbg_qt[:], qt[:])

            # ---------------- Phase B: attention + proj ----------------
            with (
                tc.tile_pool(name="ebig", bufs=2) as ebp,
                tc.tile_pool(name="zr", bufs=3) as zrp,
                tc.tile_pool(name="mstr", bufs=3) as mstr,
                tc.tile_pool(name="vstr", bufs=3) as vstr,
                tc.tile_pool(name="at", bufs=2) as atp,
                tc.tile_pool(name="wop", bufs=2) as wop,
                tc.tile_pool(name="outp", bufs=2) as outp,
                tc.tile_pool(name="pss", bufs=3, space="PSUM") as pss,
                tc.tile_pool(name="psav", bufs=1, space="PSUM") as psav,
                tc.tile_pool(name="pspj", bufs=1, space="PSUM") as pspj,
            ):
                for qh in range(2):
                    av = [psav.tile([128, 512], FP, tag=f"av{g}",
                                    name=f"av{g}")
                          for g in range(4)]
                    # One accumulation group per PSUM bank: start=True zeroes
                    # the WHOLE bank (2KB zero-region), so a single zeroing
                    # matmul opens each bank; all AV matmuls accumulate.
                    for g in range(4):
                        nc.tensor.matmul(av[g][:], ones_sb[:1, :],
                                         zeros_sb[:1, :], start=True,
                                         stop=False, skip_group_check=True)
                    for c in range(NKC):
                        mt = mstr.tile([128, QH], FP, tag="mt")
                        nc.sync.dma_start(
                            mt[:], mask_d[c, :, qh * QH:(qh + 1) * QH])
                        vch = vstr.tile([128, E], FP, tag="vch")
                        nc.sync.dma_start(
                            vch[:], v_dram[c * 128:(c + 1) * 128, :])
                        eb = ebp.tile([128, H, QH], FP, tag="eb")
                        for h in range(H):
                            po = (h % 2) * 64
                            ps = pss.tile([128, QH], FP, tag="ps")
                            nc.tensor.matmul(
                                ps[:],
                                kt[po:po + 64, h // 2,
                                   c * 128:(c + 1) * 128],
                                qt[po:po + 64, h // 2,
                                   qh * QH:(qh + 1) * QH],
                                start=True, stop=True)
                            nc.vector.tensor_mul(ps[:], ps[:], mt[:])
                            nc.scalar.activation(eb[:, h, :], ps[:], AF.Exp)
                        # Z = sum over heads (chained adds), R = 1/Z
                        z = zrp.tile([128, QH], FP, tag="z")
                        nc.vector.tensor_add(z[:], eb[:, 0, :], eb[:, 1, :])
                        for h in range(2, H):
                            nc.vector.tensor_add(z[:], z[:], eb[:, h, :])
                        r = zrp.tile([128, QH], FP, tag="r")
                        nc.vector.reciprocal(r[:], z[:])
                        if debug_taps and qh == 0 and c == 2:
                            nc.sync.dma_start(dbg_eb[:], eb[:])
                        # normalize in place then AV
                        for h in range(H):
                            nc.vector.tensor_mul(eb[:, h, :], eb[:, h, :],
                                                 r[:])
                        if debug_taps and qh == 0 and c == 2:
                            nc.sync.dma_start(dbg_p[:], eb[:])
                        for h in range(H):
                            g, g2, po = h // 4, (h // 2) % 2, (h % 2) * 64
                            nc.tensor.matmul(
                                av[g][po:po + 64, g2 * QH:(g2 + 1) * QH],
                                vch[:, h * 64:(h + 1) * 64], eb[:, h, :],
                                start=False, stop=False,
                                skip_group_check=True)
                    # Close each bank's accumulation group (adds zeros).
                    for g in range(4):
                        nc.tensor.matmul(av[g][:], ones_sb[:1, :],
                                         zeros_sb[:1, :], start=False,
                                         stop=True, skip_group_check=True)
                    # aT and projection for this q-half
                    at = atp.tile([128, 8, QH], FP, tag="at")
                    for j in range(8):
                        nc.scalar.activation(
                            at[:, j, :],
                            av[j // 2][:, (j % 2) * QH:(j % 2 + 1) * QH],
                            AF.Copy)
                    if debug_taps and qh == 0:
                        nc.sync.dma_start(dbg_at[:], at[:])
                    for cc in range(2):
                        wocc = wop.tile([128, 8, 512], FP, tag="wocc")
                        nc.sync.dma_start(
                            wocc[:],
                            wo_d[:, cc * 512:(cc + 1) * 512].rearrange(
                                "(e p) c -> p e c", p=128))
                        for qs in range(2):
                            ps = pspj.tile([128, 512], FP, tag="pj")
                            for e in range(8):
                                nc.tensor.matmul(
                                    ps[:], at[:, e, qs * 128:(qs + 1) * 128],
                                    wocc[:, e, :], start=(e == 0), stop=False)
                            nc.tensor.matmul(
                                ps[:], ones_sb[:1, :],
                                bo_sb[:1, cc * 512:(cc + 1) * 512],
                                start=False, stop=True)
                            ot = outp.tile([128, 512], FP, tag="ot")
                            nc.scalar.activation(ot[:], ps[:], AF.Copy)
                            nc.sync.dma_start(
                                out_d[qh * QH + qs * 128:
                                      qh * QH + (qs + 1) * 128,
                                      cc * 512:(cc + 1) * 512], ot[:])

            if loop is not None:
                loop.__exit__(None, None, None)

    nc.compile()
    return nc


def prep_inputs(x, W_atten, b_atten, W_proj, b_proj):
    """Host-side prep: per-core input dicts (numpy, fp32)."""
    x = np.asarray(x, dtype=np.float32)
    W3 = np.asarray(W_atten, dtype=np.float32).reshape(E, H, 3, HD)
    b3 = np.asarray(b_atten, dtype=np.float32).reshape(H, 3, HD)
    wq = np.ascontiguousarray(W3[:, :, 0, :].reshape(E, E))
    wk = np.ascontiguousarray(W3[:, :, 1, :].reshape(E, E))
    wv = np.ascontiguousarray(W3[:, :, 2, :].reshape(E, E))
    bq = np.ascontiguousarray(b3[:, 0, :].reshape(E).reshape(8, 128).T)
    bk = np.ascontiguousarray(b3[:, 1, :].reshape(E).reshape(8, 128).T)
    bv = b3[:, 2, :].reshape(1, E).copy()
    wo = np.asarray(W_proj, dtype=np.float32)
    bo = np.asarray(b_proj, dtype=np.float32).reshape(1, E).copy()

    in_maps = []
    for core in range(N_CORES):
        b, qb = core // 4, core % 4
        xT = np.ascontiguousarray(x[b].T)
        xTq = np.ascontiguousarray(x[b, qb * DQ:(qb + 1) * DQ, :].T)
        qi = qb * DQ + np.arange(DQ)[None, None, :]       # global q index
        kj = (np.arange(NKC)[:, None, None] * KC
              + np.arange(KC)[None, :, None])             # global key index
        mask = (qi >= kj).astype(np.float32)              # [NKC, KC, DQ]
        in_maps.append({
            "xT": xT, "xTq": xTq,
            "wq": wq, "wk": wk, "wv": wv, "wo": wo,
            "bq": bq, "bk": bk, "bv": bv, "bo": bo,
            "maskT": np.ascontiguousarray(mask),
        })
    return in_maps


def kernel(x, W_atten, b_atten, W_proj, b_proj):
    nc = build_program(reps=1)
    in_maps = prep_inputs(x, W_atten, b_atten, W_proj, b_proj)
    res = bass_utils.run_bass_kernel_spmd(
        nc, in_maps, core_ids=list(range(N_CORES)))
    out = np.empty((B, S, E), dtype=np.float32)
    for core in range(N_CORES):
        b, qb = core // 4, core % 4
        out[b, qb * DQ:(qb + 1) * DQ, :] = res.results[core]["out"]
    return out
